# revision 13
# baseline (speedup 1.0000x reference)
"""DCNv2 (deformable conv v2) forward on 8 Trainium2 NeuronCores.

Problem (hardcoded): x [4,128,64,64] f32, offset_w [27,128,3,3], offset_b [27],
dcn_w [128,128,3,3]; STRIDE=1, PAD=1, K=3. Output [4,128,64,64] f32.

Sharding: data-parallel — 8 cores = 4 images x 2 row-halves (32 output rows each).

Per-core pipeline (one Bass/Tile program, SPMD):
  1. offset conv: 9 tap-matmuls on PE over the channel-major padded image
     -> om [27, 2048] (channel rows permuted: 0-8 off_y, 9-17 off_x, 18-26 mask).
  2. PE-transpose om to pos-major [128 pos, 27] blocks; coef math on DVE/ACT:
     pos=om+grid, clip, frac=mod(pos,1), floor, sigmoid(mask), bilinear coefs
     c1..c4 (mask folded in), patch index = y1*67+x1 (int32).
  3. indirect DMA gather: per pos a 2KB patch (2x2 pixels x 128ch) from a
     host-prebuilt patch array [4489, 512] in DRAM -> [128 pos, 9 taps, 512].
  4. combine on DVE/GPSIMD with per-partition (=per-pos) scalars:
     value = c1*v1 + c3*v3 + c2*v2 + c4*v4  (pos-major [128, 128ch]).
  5. PE-transpose value to channel-major, GEMM with dcn_w taps accumulating in
     PSUM -> out [128 oc, 128 pos] blocks -> DMA to DRAM (NCHW directly).
"""

import numpy as np

import concourse.bacc as bacc
import concourse.bass as bass
import concourse.mybir as mybir
import concourse.tile as tile
from concourse.bass import IndirectOffsetOnAxis
from concourse.bass_utils import run_bass_kernel_spmd

N, C, H, W = 4, 128, 64, 64
OC = 128
PH = H + 3          # padded side: 67  (pad 1 top/left, 2 bottom/right)
NPIX = PH * PH      # 4489
NCORES = 8
ROWS = H // 2       # output rows per core: 32
NPOS = ROWS * W     # 2048
NBLK = NPOS // 128  # 16 pos-blocks of 128
NCHUNK = 4          # conv chunks of 512 positions (8 output rows)
F32 = mybir.dt.float32
AF = mybir.ActivationFunctionType
ALU = mybir.AluOpType

_cache = {}


def _build_program():
    nc = bacc.Bacc(
        "TRN2", target_bir_lowering=False, debug=False, num_devices=NCORES
    )

    padcm_d = nc.dram_tensor("padcm", [C, PH, PH], F32, kind="ExternalInput")
    patches_d = nc.dram_tensor("patches", [NPIX, 4 * C], F32, kind="ExternalInput")
    offw_d = nc.dram_tensor("offw", [C, 9, 32], F32, kind="ExternalInput")
    dcnw_d = nc.dram_tensor("dcnw", [C, 9, OC], F32, kind="ExternalInput")
    grid_d = nc.dram_tensor("gridpm", [128, NBLK, 27], F32, kind="ExternalInput")
    ident_d = nc.dram_tensor("ident", [128, 128], F32, kind="ExternalInput")
    out_d = nc.dram_tensor("out_cm", [OC, NPOS], F32, kind="ExternalOutput")

    with tile.TileContext(nc) as tc:
        with (
            tc.tile_pool(name="const", bufs=1) as cpool,
            tc.tile_pool(name="omsb", bufs=1) as ompool,
            tc.tile_pool(name="meta", bufs=1) as mpool,
            tc.tile_pool(name="gath", bufs=2) as gpool,
            tc.tile_pool(name="val", bufs=4) as vpool,
            tc.tile_pool(name="valT", bufs=12) as vtpool,
            tc.tile_pool(name="outsb", bufs=2) as opool,
            tc.tile_pool(name="ps_conv", bufs=2, space="PSUM") as ps_conv,
            tc.tile_pool(name="ps_omt", bufs=2, space="PSUM") as ps_omt,
            tc.tile_pool(name="ps_valt", bufs=2, space="PSUM") as ps_valt,
            tc.tile_pool(name="ps_gemm", bufs=2, space="PSUM") as ps_gemm,
        ):
            # ---- constants into SBUF ----
            padcm = cpool.tile([C, PH, PH], F32)
            offw = cpool.tile([C, 9, 32], F32)
            dcnw = cpool.tile([C, 9, OC], F32)
            grid = cpool.tile([128, NBLK, 27], F32)
            ident = cpool.tile([128, 128], F32)
            nc.sync.dma_start(padcm[:], padcm_d.ap())
            nc.sync.dma_start(offw[:], offw_d.ap())
            nc.sync.dma_start(dcnw[:], dcnw_d.ap())
            nc.sync.dma_start(grid[:], grid_d.ap())
            nc.sync.dma_start(ident[:], ident_d.ap())

            # ---- work tiles ----
            omsb = [
                ompool.tile([27, 512], F32, tag=f"om{q}", name=f"omsb{q}")
                for q in range(NCHUNK)
            ]
            OMP = mpool.tile([128, NBLK, 27], F32)
            FR = mpool.tile([128, NBLK, 18], F32)
            FL = mpool.tile([128, NBLK, 18], F32)
            ITMP = mpool.tile([128, NBLK, 18], mybir.dt.int32)
            META = mpool.tile([128, NBLK, 45], F32)
            IDX = mpool.tile([128, NBLK, 9], mybir.dt.int32)

            # ---- 1. offset conv (chunks of 8 output rows = 512 pos) ----
            # core's first output row is h0 = 32*(core%2); the padded image is
            # full, so row offset = h0 + 8*q + dy.  h0 differs per core, but the
            # PROGRAM must be identical across cores (SPMD).  Trick: bake h0=0
            # and ship a per-core padded image that is pre-shifted: for the
            # lower half we pass the image shifted by 32 rows (host prep).
            for q in range(NCHUNK):
                ps = ps_conv.tile([27, 512], F32, tag="psconv")
                for t in range(9):
                    dy, dx = t // 3, t % 3
                    rhs = padcm[:, 8 * q + dy:8 * q + dy + 8, dx:dx + W]
                    nc.tensor.matmul(
                        ps[:], offw[:, t, 0:27], rhs,
                        start=(t == 0), stop=(t == 8),
                    )
                nc.scalar.copy(omsb[q][:], ps[:])

            # ---- 2. om -> pos-major + coef math ----
            for b in range(NBLK):
                q, r = b // 4, b % 4
                pt = ps_omt.tile([128, 27], F32, tag="psomt")
                nc.tensor.transpose(
                    pt[:], omsb[q][:, 128 * r:128 * (r + 1)], ident[0:27, 0:27]
                )
                nc.scalar.copy(OMP[:, b, :], pt[:])

            nc.vector.tensor_tensor(OMP[:], OMP[:], grid[:], ALU.add)
            nc.vector.tensor_scalar_max(OMP[:, :, 0:18], OMP[:, :, 0:18], 0.0)
            nc.vector.tensor_scalar_min(OMP[:, :, 0:18], OMP[:, :, 0:18], 65.0)
            nc.scalar.activation(OMP[:, :, 18:27], OMP[:, :, 18:27], AF.Sigmoid)
            # floor(pos) for pos>=0, robust to cast rounding mode:
            #   t = f32(int32(pos)); t -= (t > pos); frac = pos - t
            nc.vector.tensor_copy(ITMP[:], OMP[:, :, 0:18])
            nc.vector.tensor_copy(FL[:], ITMP[:])
            nc.vector.tensor_tensor(FR[:], FL[:], OMP[:, :, 0:18], ALU.is_gt)
            nc.vector.tensor_sub(FL[:], FL[:], FR[:])
            nc.vector.tensor_sub(FR[:], OMP[:, :, 0:18], FL[:])
            # c3 = fy*m ; c1 = m - c3 ; c2 = c1*fx ; c1 -= c2 ; c4 = c3*fx ; c3 -= c4
            nc.vector.tensor_tensor(META[:, :, 18:27], FR[:, :, 0:9], OMP[:, :, 18:27], ALU.mult)
            nc.vector.tensor_sub(META[:, :, 0:9], OMP[:, :, 18:27], META[:, :, 18:27])
            nc.vector.tensor_tensor(META[:, :, 9:18], META[:, :, 0:9], FR[:, :, 9:18], ALU.mult)
            nc.vector.tensor_sub(META[:, :, 0:9], META[:, :, 0:9], META[:, :, 9:18])
            nc.vector.tensor_tensor(META[:, :, 27:36], META[:, :, 18:27], FR[:, :, 9:18], ALU.mult)
            nc.vector.tensor_sub(META[:, :, 18:27], META[:, :, 18:27], META[:, :, 27:36])
            nc.vector.scalar_tensor_tensor(
                META[:, :, 36:45], FL[:, :, 0:9], 67.0, FL[:, :, 9:18],
                ALU.mult, ALU.add,
            )
            nc.vector.tensor_copy(IDX[:], META[:, :, 36:45])

            # ---- 3..5 per pos-block ----
            for b in range(NBLK):
                g = gpool.tile([128, 9, 4 * C], F32, tag="gath")
                for k in range(9):
                    nc.gpsimd.indirect_dma_start(
                        out=g[:, k, :],
                        out_offset=None,
                        in_=patches_d.ap(),
                        in_offset=IndirectOffsetOnAxis(ap=IDX[:, b, k:k + 1], axis=0),
                    )
                vts = []
                for k in range(9):
                    v = vpool.tile([128, C], F32, tag="val")
                    eng = nc.vector
                    eng.tensor_scalar_mul(v[:], g[:, k, 0:128], META[:, b, 0 + k:1 + k])
                    eng.scalar_tensor_tensor(
                        v[:], g[:, k, 128:256], META[:, b, 18 + k:19 + k], v[:],
                        ALU.mult, ALU.add)
                    eng.scalar_tensor_tensor(
                        v[:], g[:, k, 256:384], META[:, b, 9 + k:10 + k], v[:],
                        ALU.mult, ALU.add)
                    eng.scalar_tensor_tensor(
                        v[:], g[:, k, 384:512], META[:, b, 27 + k:28 + k], v[:],
                        ALU.mult, ALU.add)
                    pv = ps_valt.tile([128, C], F32, tag="psvalt")
                    nc.tensor.transpose(pv[:], v[:], ident[:])
                    vt = vtpool.tile([C, 128], F32, tag="valT")
                    nc.scalar.copy(vt[:], pv[:])
                    vts.append(vt)
                pg = ps_gemm.tile([OC, 128], F32, tag="psgemm")
                for k in range(9):
                    nc.tensor.matmul(
                        pg[:], dcnw[:, k, :], vts[k][:],
                        start=(k == 0), stop=(k == 8),
                    )
                ob = opool.tile([OC, 128], F32, tag="outsb")
                nc.scalar.copy(ob[:], pg[:])
                nc.sync.dma_start(out_d.ap()[:, 128 * b:128 * (b + 1)], ob[:])

    nc.compile()
    return nc


def _host_prep(x, offset_w, offset_b, dcn_w):
    """Build per-core input maps (pure data layout, no math on x)."""
    x = np.ascontiguousarray(np.asarray(x, np.float32))
    offset_w = np.asarray(offset_w, np.float32)
    offset_b = np.asarray(offset_b, np.float32)
    dcn_w = np.asarray(dcn_w, np.float32)

    padx = np.pad(x, ((0, 0), (0, 0), (1, 2), (1, 2)))          # [N,C,67,67]
    P68 = np.pad(padx.transpose(0, 2, 3, 1), ((0, 0), (0, 1), (0, 1), (0, 0)))
    patches = np.ascontiguousarray(np.concatenate(
        [P68[:, :PH, :PH], P68[:, 1:, :PH], P68[:, :PH, 1:], P68[:, 1:, 1:]],
        axis=-1).reshape(N, NPIX, 4 * C))

    perm = np.array([2 * j for j in range(9)]
                    + [2 * j + 1 for j in range(9)] + list(range(18, 27)))
    offw_lhsT = np.zeros((C, 9, 32), np.float32)
    offw_lhsT[:, :, 0:27] = offset_w[perm].transpose(2, 3, 1, 0).reshape(9, C, 27).transpose(1, 0, 2)
    dcnw_t = np.ascontiguousarray(
        dcn_w.transpose(2, 3, 1, 0).reshape(9, C, OC).transpose(1, 0, 2))

    iy = np.repeat(np.arange(3) - 1, 3).astype(np.float32)
    ix = np.tile(np.arange(3) - 1, 3).astype(np.float32)
    ident = np.eye(128, dtype=np.float32)
    bperm = offset_b[perm]

    in_maps = []
    for core in range(NCORES):
        n, half = core // 2, core % 2
        h0 = half * ROWS
        # per-core grid, pos-major [128, NBLK, 27]; block b = 2 output rows
        p_idx = np.arange(NPOS)
        hh_ = (h0 + p_idx // W).astype(np.float32)
        ww_ = (p_idx % W).astype(np.float32)
        grid = np.zeros((NPOS, 27), np.float32)
        grid[:, 0:9] = hh_[:, None] + 1.0 + iy[None, :] + bperm[0:9][None, :]
        grid[:, 9:18] = ww_[:, None] + 1.0 + ix[None, :] + bperm[9:18][None, :]
        grid[:, 18:27] = bperm[18:27][None, :]
        gridpm = np.ascontiguousarray(
            grid.reshape(NBLK, 128, 27).transpose(1, 0, 2))
        # conv trick: the SPMD program reads conv rows 8q+dy from row 0; ship
        # the padded image shifted so local row 0 == padded row h0 (conv only
        # reads rows [0, 34)).  The gather uses patches_d, never shifted.
        pc = padx[n]
        if h0:
            pc = np.ascontiguousarray(pc[:, h0:, :])
            pc = np.pad(pc, ((0, 0), (0, h0), (0, 0)))
        in_maps.append({
            "padcm": np.ascontiguousarray(pc),
            "patches": patches[n],
            "offw": offw_lhsT,
            "dcnw": dcnw_t,
            "gridpm": gridpm,
            "ident": ident,
        })
    return in_maps


def kernel(x, offset_w, offset_b, dcn_w):
    if "nc" not in _cache:
        _cache["nc"] = _build_program()
    nc = _cache["nc"]
    in_maps = _host_prep(x, offset_w, offset_b, dcn_w)
    res = run_bass_kernel_spmd(nc, in_maps, core_ids=list(range(NCORES)))
    out = np.zeros((N, C, H, W), np.float32)
    for core in range(NCORES):
        n, half = core // 2, core % 2
        h0 = half * ROWS
        oc = res.results[core]["out_cm"]
        out[n, :, h0:h0 + ROWS, :] = oc.reshape(OC, ROWS, W)
    return out


# revision 31
# speedup vs baseline: 480.9632x; 480.9632x over previous
"""DCNv2 (deformable conv v2) forward on 8 Trainium2 NeuronCores.

Problem (hardcoded): x [4,128,64,64] f32, offset_w [27,128,3,3], offset_b [27],
dcn_w [128,128,3,3]; STRIDE=1, PAD=1, K=3. Output [4,128,64,64] f32.

Sharding: data-parallel — 8 cores = 4 images x 2 row-halves (32 output rows each).

Per-core pipeline (one Bass/Tile program, SPMD):
  1. offset conv: 9 tap-matmuls on PE over the channel-major padded image
     -> om [27, 2048] (channel rows permuted: 0-8 off_y, 9-17 off_x, 18-26 mask).
  2. PE-transpose om to pos-major [128 pos, 27] blocks; coef math on DVE/ACT:
     pos=om+grid, clip, frac=mod(pos,1), floor, sigmoid(mask), bilinear coefs
     c1..c4 (mask folded in), patch index = y1*67+x1 (int32).
  3. indirect DMA gather (SWDGE, one idx per partition, dual qPoolDynamic
     queues): per pos a 2KB patch (2x2 pixels x 128ch) from a host-prebuilt
     patch array [4489, 512] in DRAM -> [128 pos, 9 taps, 512].
  4. combine on DVE with per-partition (=per-pos) scalar FMAs:
     value = c1*v1 + c3*v3 + c2*v2 + c4*v4  (pos-major [128, 128ch]).
  5. PE-transpose value to channel-major, GEMM with dcn_w taps accumulating in
     PSUM (block pairs, N=256) -> out [oc, pos] -> DMA to DRAM (NCHW directly).

The conv is chunked (first chunk = 2 output rows) so the gather stream — the
per-core bottleneck (144 SWDGE indirect DMAs + 37.7 MB of patch traffic) —
starts as early as possible and everything else hides under it.
"""

import numpy as np

import concourse.bacc as bacc
import concourse.bass as bass
import concourse.mybir as mybir
import concourse.tile as tile
from concourse.bass import IndirectOffsetOnAxis
from concourse.bass_utils import run_bass_kernel_spmd

N, C, H, W = 4, 128, 64, 64
OC = 128
PH = H + 3          # padded side: 67  (pad 1 top/left, 2 bottom/right)
NPIX = PH * PH      # 4489
NCORES = 8
ROWS = H // 2       # output rows per core: 32
NPOS = ROWS * W     # 2048
NBLK = NPOS // 128  # 16 pos-blocks of 128
NCHUNK = 4          # conv chunks of 512 positions (8 output rows)
F32 = mybir.dt.float32
F32R = mybir.dt.float32r
AF = mybir.ActivationFunctionType
ALU = mybir.AluOpType

_cache = {}


def _build_program():
    nc = bacc.Bacc(
        "TRN2", target_bir_lowering=False, debug=False, num_devices=NCORES,
        num_swdge_queues=2,
    )

    padcm_d = nc.dram_tensor("padcm", [C, PH, PH], F32, kind="ExternalInput")
    patches_d = nc.dram_tensor("patches", [NPIX, 4 * C], F32, kind="ExternalInput")
    offw_d = nc.dram_tensor("offw", [C, 9, 32], F32, kind="ExternalInput")
    dcnw_d = nc.dram_tensor("dcnw", [C, 9, OC], F32, kind="ExternalInput")
    grid_d = nc.dram_tensor("gridpm", [128, NBLK, 27], F32, kind="ExternalInput")
    ident_d = nc.dram_tensor("ident", [128, 128], F32, kind="ExternalInput")
    out_d = nc.dram_tensor("out_cm", [OC, NPOS], F32, kind="ExternalOutput")

    with tile.TileContext(nc) as tc:
        with (
            tc.tile_pool(name="const", bufs=1) as cpool,
            tc.tile_pool(name="omsb", bufs=1) as ompool,
            tc.tile_pool(name="meta", bufs=1) as mpool,
            tc.tile_pool(name="gath", bufs=3) as gpool,
            tc.tile_pool(name="val", bufs=4) as vpool,
            tc.tile_pool(name="valT", bufs=12) as vtpool,
            tc.tile_pool(name="outsb", bufs=2) as opool,
            tc.tile_pool(name="ps_conv", bufs=2, space="PSUM") as ps_conv,
            tc.tile_pool(name="ps_omt", bufs=2, space="PSUM") as ps_omt,
            tc.tile_pool(name="ps_valt", bufs=2, space="PSUM") as ps_valt,
            tc.tile_pool(name="ps_gemm", bufs=2, space="PSUM") as ps_gemm,
        ):
            # ---- constants into SBUF ----
            padcm = cpool.tile([C, PH, PH], F32)
            offw = cpool.tile([C, 9, 32], F32)
            dcnw = cpool.tile([C, 9, OC], F32)
            grid = cpool.tile([128, NBLK, 27], F32)
            ident = cpool.tile([128, 128], F32)
            nc.sync.dma_start(offw[:], offw_d.ap())
            nc.sync.dma_start(ident[:], ident_d.ap())
            nc.sync.dma_start(grid[:], grid_d.ap())
            # padcm in chunk-sized slabs so conv chunk 0 starts early
            pflat_s = padcm[:].rearrange("c a b -> c (a b)")
            pflat_d = padcm_d.ap().rearrange("c a b -> c (a b)")
            bounds = [0, 5, 11, 19, 27, 35, PH]
            for r0, r1 in zip(bounds[:-1], bounds[1:]):
                nc.sync.dma_start(
                    pflat_s[:, r0 * PH:r1 * PH], pflat_d[:, r0 * PH:r1 * PH]
                )
            nc.sync.dma_start(dcnw[:], dcnw_d.ap())

            # ---- work tiles ----
            CHUNKS = [(0, 2), (2, 6), (8, 8), (16, 8), (24, 8)]
            CBLK = [nr // 2 for _, nr in CHUNKS]        # blocks per chunk
            CB0 = [sum(CBLK[:i]) for i in range(len(CHUNKS))]  # first block
            omsb = [
                ompool.tile([27, nr * W], F32, tag=f"om{q}", name=f"omsb{q}")
                for q, (_, nr) in enumerate(CHUNKS)
            ]
            OMPq = [mpool.tile([128, CBLK[q], 27], F32, name=f"OMP{q}") for q in range(len(CHUNKS))]
            FRq = [mpool.tile([128, CBLK[q], 18], F32, name=f"FR{q}") for q in range(len(CHUNKS))]
            FLq = [mpool.tile([128, CBLK[q], 18], F32, name=f"FL{q}") for q in range(len(CHUNKS))]
            ITMPq = [mpool.tile([128, CBLK[q], 18], mybir.dt.int32, name=f"ITMP{q}")
                     for q in range(len(CHUNKS))]
            METAq = [mpool.tile([128, CBLK[q], 45], F32, name=f"META{q}") for q in range(len(CHUNKS))]
            IDXq = [mpool.tile([128, CBLK[q], 9], mybir.dt.int32, name=f"IDX{q}")
                    for q in range(len(CHUNKS))]

            # ---- 1. offset conv (chunks of 8 output rows = 512 pos) ----
            # core's first output row is h0 = 32*(core%2); the padded image is
            # full, so row offset = h0 + 8*q + dy.  h0 differs per core, but the
            # PROGRAM must be identical across cores (SPMD).  Trick: bake h0=0
            # and ship a per-core padded image that is pre-shifted: for the
            # lower half we pass the image shifted by 32 rows (host prep).
            for q, (row0, nrows) in enumerate(CHUNKS):
                ncols = nrows * W
                ps = ps_conv.tile([27, 512], F32, tag="psconv")
                for t in range(9):
                    dy, dx = t // 3, t % 3
                    rhs = padcm[:, row0 + dy:row0 + dy + nrows, dx:dx + W]
                    nc.tensor.matmul(
                        ps[:, 0:ncols], offw[:, t, 0:27], rhs,
                        start=(t == 0), stop=(t == 8),
                    )
                nc.scalar.copy(omsb[q][:, 0:ncols], ps[:, 0:ncols])

                # ---- 2. om -> pos-major + coef math (per conv chunk) ----
                OMP, FR, FL = OMPq[q], FRq[q], FLq[q]
                ITMP, META, IDX = ITMPq[q], METAq[q], IDXq[q]
                for r in range(CBLK[q]):
                    pt = ps_omt.tile([128, 27], F32, tag="psomt")
                    nc.tensor.transpose(
                        pt[:], omsb[q][:, 128 * r:128 * (r + 1)],
                        ident[0:27, 0:27]
                    )
                    nc.scalar.copy(OMP[:, r, :], pt[:])

                B = slice(0, CBLK[q])
                gridB = grid[:, CB0[q]:CB0[q] + CBLK[q], :]
                nc.vector.tensor_tensor(OMP[:, B, :], OMP[:, B, :], gridB, ALU.add)
                nc.vector.tensor_scalar_max(OMP[:, B, 0:18], OMP[:, B, 0:18], 0.0)
                nc.vector.tensor_scalar_min(OMP[:, B, 0:18], OMP[:, B, 0:18], 65.0)
                nc.scalar.activation(OMP[:, B, 18:27], OMP[:, B, 18:27], AF.Sigmoid)
                # floor(pos) for pos>=0, robust to cast rounding mode:
                #   t = f32(int32(pos)); t -= (t > pos); frac = pos - t
                nc.vector.tensor_copy(ITMP[:, B, :], OMP[:, B, 0:18])
                nc.vector.tensor_copy(FL[:, B, :], ITMP[:, B, :])
                nc.vector.tensor_tensor(FR[:, B, :], FL[:, B, :], OMP[:, B, 0:18], ALU.is_gt)
                nc.vector.tensor_sub(FL[:, B, :], FL[:, B, :], FR[:, B, :])
                nc.vector.tensor_sub(FR[:, B, :], OMP[:, B, 0:18], FL[:, B, :])
                # c3 = fy*m; c1 = m-c3; c2 = c1*fx; c1 -= c2; c4 = c3*fx; c3 -= c4
                nc.vector.tensor_tensor(META[:, B, 18:27], FR[:, B, 0:9], OMP[:, B, 18:27], ALU.mult)
                nc.vector.tensor_sub(META[:, B, 0:9], OMP[:, B, 18:27], META[:, B, 18:27])
                nc.vector.tensor_tensor(META[:, B, 9:18], META[:, B, 0:9], FR[:, B, 9:18], ALU.mult)
                nc.vector.tensor_sub(META[:, B, 0:9], META[:, B, 0:9], META[:, B, 9:18])
                nc.vector.tensor_tensor(META[:, B, 27:36], META[:, B, 18:27], FR[:, B, 9:18], ALU.mult)
                nc.vector.tensor_sub(META[:, B, 18:27], META[:, B, 18:27], META[:, B, 27:36])
                nc.vector.scalar_tensor_tensor(
                    META[:, B, 36:45], FL[:, B, 0:9], 67.0, FL[:, B, 9:18],
                    ALU.mult, ALU.add,
                )
                nc.vector.tensor_copy(IDX[:, B, :], META[:, B, 36:45])

            # ---- 3..5 per pos-block pair (GEMM N=256 for full-rate f32r) ----
            for j in range(NBLK // 2):
                vts = [vtpool.tile([C, 256], F32, tag="valT", name=f"vt{j}_{k}")
                       for k in range(9)]
                for half in range(2):
                    b = 2 * j + half
                    cq = next(i for i in range(len(CHUNKS))
                              if CB0[i] <= b < CB0[i] + CBLK[i])
                    META, IDX = METAq[cq], IDXq[cq]
                    br = b - CB0[cq]
                    g = gpool.tile([128, 9, 4 * C], F32, tag="gath")
                    for k in range(9):
                        gi = nc.gpsimd.indirect_dma_start(
                            out=g[:, k, :],
                            out_offset=None,
                            in_=patches_d.ap(),
                            in_offset=IndirectOffsetOnAxis(
                                ap=IDX[:, br, k:k + 1], axis=0),
                        )
                        # alternate SWDGE queues so desc-gen runs on both
                        # Q7 core pairs in parallel
                        if k % 2 and _cache.get("use_q1", True):
                            gi.ins.queue = "qPoolDynamic1"
                    for k in range(9):
                        v = vpool.tile([128, C], F32, tag="val")
                        eng = nc.vector
                        eng.tensor_scalar_mul(v[:], g[:, k, 0:128], META[:, br, 0 + k:1 + k])
                        eng.scalar_tensor_tensor(
                            v[:], g[:, k, 128:256], META[:, br, 18 + k:19 + k], v[:],
                            ALU.mult, ALU.add)
                        eng.scalar_tensor_tensor(
                            v[:], g[:, k, 256:384], META[:, br, 9 + k:10 + k], v[:],
                            ALU.mult, ALU.add)
                        eng.scalar_tensor_tensor(
                            v[:], g[:, k, 384:512], META[:, br, 27 + k:28 + k], v[:],
                            ALU.mult, ALU.add)
                        pv = ps_valt.tile([128, C], F32, tag="psvalt")
                        nc.tensor.transpose(pv[:], v[:], ident[:])
                        nc.scalar.copy(vts[k][:, 128 * half:128 * (half + 1)], pv[:])
                pg = ps_gemm.tile([OC, 256], F32, tag="psgemm")
                for k in range(9):
                    nc.tensor.matmul(
                        pg[:], dcnw[:, k, :], vts[k][:],
                        start=(k == 0), stop=(k == 8),
                    )
                ob = opool.tile([OC, 256], F32, tag="outsb")
                nc.scalar.copy(ob[:], pg[:])
                nc.sync.dma_start(out_d.ap()[:, 256 * j:256 * (j + 1)], ob[:])

    nc.compile()
    return nc


def _host_prep(x, offset_w, offset_b, dcn_w):
    """Build per-core input maps (pure data layout, no math on x)."""
    x = np.ascontiguousarray(np.asarray(x, np.float32))
    offset_w = np.asarray(offset_w, np.float32)
    offset_b = np.asarray(offset_b, np.float32)
    dcn_w = np.asarray(dcn_w, np.float32)

    padx = np.pad(x, ((0, 0), (0, 0), (1, 2), (1, 2)))          # [N,C,67,67]
    P68 = np.pad(padx.transpose(0, 2, 3, 1), ((0, 0), (0, 1), (0, 1), (0, 0)))
    patches = np.ascontiguousarray(np.concatenate(
        [P68[:, :PH, :PH], P68[:, 1:, :PH], P68[:, :PH, 1:], P68[:, 1:, 1:]],
        axis=-1).reshape(N, NPIX, 4 * C))

    perm = np.array([2 * j for j in range(9)]
                    + [2 * j + 1 for j in range(9)] + list(range(18, 27)))
    offw_lhsT = np.zeros((C, 9, 32), np.float32)
    offw_lhsT[:, :, 0:27] = offset_w[perm].transpose(2, 3, 1, 0).reshape(9, C, 27).transpose(1, 0, 2)
    dcnw_t = np.ascontiguousarray(
        dcn_w.transpose(2, 3, 1, 0).reshape(9, C, OC).transpose(1, 0, 2))

    iy = np.repeat(np.arange(3) - 1, 3).astype(np.float32)
    ix = np.tile(np.arange(3) - 1, 3).astype(np.float32)
    ident = np.eye(128, dtype=np.float32)
    bperm = offset_b[perm]

    in_maps = []
    for core in range(NCORES):
        n, half = core // 2, core % 2
        h0 = half * ROWS
        # per-core grid, pos-major [128, NBLK, 27]; block b = 2 output rows
        p_idx = np.arange(NPOS)
        hh_ = (h0 + p_idx // W).astype(np.float32)
        ww_ = (p_idx % W).astype(np.float32)
        grid = np.zeros((NPOS, 27), np.float32)
        grid[:, 0:9] = hh_[:, None] + 1.0 + iy[None, :] + bperm[0:9][None, :]
        grid[:, 9:18] = ww_[:, None] + 1.0 + ix[None, :] + bperm[9:18][None, :]
        grid[:, 18:27] = bperm[18:27][None, :]
        gridpm = np.ascontiguousarray(
            grid.reshape(NBLK, 128, 27).transpose(1, 0, 2))
        # conv trick: the SPMD program reads conv rows 8q+dy from row 0; ship
        # the padded image shifted so local row 0 == padded row h0 (conv only
        # reads rows [0, 34)).  The gather uses patches_d, never shifted.
        pc = padx[n]
        if h0:
            pc = np.ascontiguousarray(pc[:, h0:, :])
            pc = np.pad(pc, ((0, 0), (0, h0), (0, 0)))
        in_maps.append({
            "padcm": np.ascontiguousarray(pc),
            "patches": patches[n],
            "offw": offw_lhsT,
            "dcnw": dcnw_t,
            "gridpm": gridpm,
            "ident": ident,
        })
    return in_maps


def kernel(x, offset_w, offset_b, dcn_w):
    if "nc" not in _cache:
        _cache["nc"] = _build_program()
    nc = _cache["nc"]
    in_maps = _host_prep(x, offset_w, offset_b, dcn_w)
    res = run_bass_kernel_spmd(nc, in_maps, core_ids=list(range(NCORES)))
    out = np.zeros((N, C, H, W), np.float32)
    for core in range(NCORES):
        n, half = core // 2, core % 2
        h0 = half * ROWS
        oc = res.results[core]["out_cm"]
        out[n, :, h0:h0 + ROWS, :] = oc.reshape(OC, ROWS, W)
    return out
